# revision 15
# baseline (speedup 1.0000x reference)
"""Trainium2 Bass kernel for nn_BiLingual (dual embedding gather + cAddTanh pool).

Computes, for two embedding tables:
    out[t, b, :] = sum_{j=0}^{S-2} tanh(W_t[idx_t[b, j]] + W_t[idx_t[b, j+1]])

Sharding: data-parallel over batch. Each of the 8 cores handles 8 batch rows
for BOTH tables (16 "row16" streams of 2048 positions); tables are replicated
(host-cast to bf16).

Design (v3: flat 4-queue gather + host-interleaved stream):

  * SWDGE dma_gather generation runs at ~7.9 ns/idx on one Q7 core pair; the
    pair is selected by queue_num.  A queue-0 call HOLDS the Pool engine for
    its whole generation while queue-1..3 calls dispatch in ~50 ns and
    generate concurrently on their own pairs, so rounds are emitted as
    [q1, q2, q3, q0]: q0's engine-block overlaps the other three pairs ->
    true 4-way parallel generation (~2 ns/idx aggregate).  (Transpose-mode
    gather is faster solo but its XBAR has only two accumulation contexts
    (queue parity) and caps at ~4 ns/idx aggregate -- measured worse.)
  * Host interleave: the gather stream for a row is row.reshape(128,16).T
    flattened, so slot g*128+p holds position 16p+g, i.e. E[p, g] =
    W[idx[16p+g]].  Consecutive positions then sit in ADJACENT GROUPS of the
    same partition, and pair formation is a single contiguous DVE add
    A[p, g] = E[p, g] + E[p, g+1] (g=0..14) -- no PE shift matmuls.
  * The 127 partition-crossing pairs (16p+15, 16p+16) are two tiny PE
    matmuls into PSUM: lhsT M1 (subdiagonal) * E[:,0,:] + lhsT I127
    (identity, [127,127]=0) * E[:,15,:].  Slot p=127 is exactly 0 =
    tanh(0), so the later reduce needs no mask.
  * ACT tanh: A (fp16) -> T_row groups 0..14; wrap PSUM -> T_row group 15.
  * Reduce: one contiguous DVE fold T8 = T[:,0:8]+T[:,8:16] (strided DVE
    reduces measured ~3x slower than contiguous ops), then a per-row PE
    ones-column matmul (fp16) sums partitions into PSUM acc[16, 8*256]
    accumulated over all 16 rows; one small strided DVE reduce folds the 8
    groups at the very end.
  * int16 index range handled by biasing: base = W[32768:], idx' =
    idx - 32768 in [-32768, 17231]; 16 trailing zero guards per row keep the
    gather ucode's trailing-negative trim from eating real indices (they
    land in the junk group 16 of the E tile).
"""
import os

import numpy as np
import ml_dtypes

from concourse import bacc, mybir
import concourse.tile as tile
from concourse.bass_utils import run_bass_kernel_spmd

P = 128
B, S, V, D = 64, 2048, 50000, 256
N_CORES = 8
B_LOC = B // N_CORES           # 8 batch rows per core
NROW = 2 * B_LOC               # 16 (table, local row) streams per core
NG = S // P                    # 16 groups per row (interleaved layout)
SPLIT = 32768
GUARD = 16                     # trailing zero-idx guards (trim protection)
NIDX = S + GUARD               # 2064 gathered slots per row
ICOL = NIDX // 16              # 129 idx columns per row
QORDER = (1, 2, 3, 0)          # queue per row within a round; q0 last

_last_results = None           # set by _run for test harness introspection


def _build_m1():
    # lhsT for out[p] = E[p+1]: lhsT[k=p+1, m=p] = 1, p <= 126
    m = np.zeros((P, P), dtype=np.float32)
    p = np.arange(P - 1)
    m[p + 1, p] = 1.0
    return m.astype(ml_dtypes.bfloat16)


def _build_i127():
    # identity with [127,127] = 0: wrap slot p=127 stays exactly 0
    m = np.eye(P, dtype=np.float32)
    m[P - 1, P - 1] = 0.0
    return m.astype(ml_dtypes.bfloat16)


def _build_red():
    # fp16 partition-reduce masks: slice r ([P, NROW] block r) has ones in
    # column r only, so lhsT.T @ t8 lands the row's partition-sum in output
    # partition r and adds 0 elsewhere (PSUM accumulation over rows).
    red = np.zeros((P, NROW * NROW), dtype=np.float16)
    for r in range(NROW):
        red[:, r * NROW + r] = 1.0
    return red


def _build_program():
    nc = bacc.Bacc(
        None,
        target_bir_lowering=False,
        dynamic_dma_scratch_size=49152,
        num_swdge_queues=4,
    )
    bf16 = mybir.dt.bfloat16
    fp16 = mybir.dt.float16
    fp32 = mybir.dt.float32
    Wp = nc.declare_dram_parameter("W_pri", [V, D], bf16, isOutput=False)
    Ws = nc.declare_dram_parameter("W_sec", [V, D], bf16, isOutput=False)
    idxA = nc.declare_dram_parameter(
        "idxA", [P, NROW * ICOL], mybir.dt.int16, isOutput=False
    )
    m1P = nc.declare_dram_parameter("m1", [P, P], bf16, isOutput=False)
    i127P = nc.declare_dram_parameter("i127", [P, P], bf16, isOutput=False)
    redP = nc.declare_dram_parameter("red", [P, NROW * NROW], fp16, isOutput=False)
    out = nc.declare_dram_parameter("out", [NROW, D], fp32, isOutput=True)

    with tile.TileContext(nc) as tc:
        with (
            tc.tile_pool(name="const", bufs=1) as const,
            tc.tile_pool(name="ebuf", bufs=8) as ebuf,
            tc.tile_pool(name="abuf", bufs=3) as abuf,
            tc.tile_pool(name="tbuf", bufs=3) as tbuf,
            tc.tile_pool(name="t8buf", bufs=3) as t8buf,
            tc.tile_pool(name="psW", bufs=2, space="PSUM") as psW,
            tc.tile_pool(name="psR", bufs=1, space="PSUM") as psR,
            tc.tile_pool(name="osb", bufs=1) as osb,
        ):
            # warm-up: tiny flat gathers, one per queue; the first pays the
            # ucode's IRAM load while the real idx table uploads.
            iZ = const.tile([P, 8], mybir.dt.int16)
            nc.gpsimd.memset(iZ[:], 0)
            eZ = const.tile([P, 1, D], bf16)
            nc.gpsimd.dma_gather(
                out_ap=eZ[:],
                in_ap=Wp[SPLIT:, :],
                idxs_ap=iZ[:, 0:1],
                num_idxs=16,
                num_idxs_reg=16,
                elem_size=D,
                queue_num=1,
            )

            iA = const.tile([P, NROW * ICOL], mybir.dt.int16)
            nc.sync.dma_start(out=iA[:], in_=idxA[:])
            m1 = const.tile([P, P], bf16)
            nc.sync.dma_start(out=m1[:], in_=m1P[:])
            i127 = const.tile([P, P], bf16)
            nc.sync.dma_start(out=i127[:], in_=i127P[:])
            red = const.tile([P, NROW * NROW], fp16)
            nc.sync.dma_start(out=red[:], in_=redP[:])

            acc = psR.tile([NROW, 8 * D], fp32, space="PSUM")

            for r in range(NROW):
                q = QORDER[r % 4]
                W = Wp if r < B_LOC else Ws
                e = ebuf.tile([P, NG + 1, D], bf16)  # 16 data groups + junk
                nc.gpsimd.dma_gather(
                    out_ap=e[:],
                    in_ap=W[SPLIT:, :],
                    idxs_ap=iA[:, r * ICOL : (r + 1) * ICOL],
                    num_idxs=NIDX,
                    num_idxs_reg=NIDX,
                    elem_size=D,
                    single_packet=False,
                    queue_num=q,
                )
                # within-partition pairs: A[p, g] = E[p, g] + E[p, g+1]
                a = abuf.tile([P, NG - 1, D], fp16)
                nc.vector.tensor_add(
                    a[:].rearrange("p g d -> p (g d)"),
                    e[:, 0 : NG - 1, :].rearrange("p g d -> p (g d)"),
                    e[:, 1:NG, :].rearrange("p g d -> p (g d)"),
                )
                # cross-partition pairs (16p+15, 16p+16) -> PSUM
                aw = psW.tile([P, D], fp32, space="PSUM")
                nc.tensor.matmul(
                    out=aw[:], lhsT=m1[:], rhs=e[:, 0, :], start=True, stop=False
                )
                nc.tensor.matmul(
                    out=aw[:], lhsT=i127[:], rhs=e[:, NG - 1, :], start=False, stop=True
                )
                t_row = tbuf.tile([P, NG, D], fp16)
                nc.scalar.activation(
                    t_row[:, 0 : NG - 1, :], a[:], mybir.ActivationFunctionType.Tanh
                )
                nc.scalar.activation(
                    t_row[:, NG - 1, :], aw[:], mybir.ActivationFunctionType.Tanh
                )
                t8 = t8buf.tile([P, 8, D], fp16)
                nc.vector.tensor_add(
                    t8[:].rearrange("p g d -> p (g d)"),
                    t_row[:, 0:8, :].rearrange("p g d -> p (g d)"),
                    t_row[:, 8:NG, :].rearrange("p g d -> p (g d)"),
                )
                # matmul free size caps at 512: 4 slices of 2 groups each
                for s in range(4):
                    nc.tensor.matmul(
                        out=acc[:, s * 2 * D : (s + 1) * 2 * D],
                        lhsT=red[:, r * NROW : (r + 1) * NROW],
                        rhs=t8[:, 2 * s : 2 * s + 2, :],
                        start=(r == 0),
                        stop=(r == NROW - 1),
                    )

            res_sb = osb.tile([NROW, D], fp32)
            nc.vector.tensor_reduce(
                out=res_sb[:],
                in_=acc[:].rearrange("p (g d) -> p d g", g=8),
                axis=mybir.AxisListType.X,
                op=mybir.AluOpType.add,
            )
            nc.sync.dma_start(out=out[:], in_=res_sb[:])

    nc.compile()
    return nc


def _host_prep(inputs_pri, inputs_sec, W_pri, W_sec):
    ip = np.asarray(inputs_pri).astype(np.int64, copy=False)
    is_ = np.asarray(inputs_sec).astype(np.int64, copy=False)
    wp = np.ascontiguousarray(
        np.asarray(W_pri, dtype=np.float32).astype(ml_dtypes.bfloat16)
    )
    ws = np.ascontiguousarray(
        np.asarray(W_sec, dtype=np.float32).astype(ml_dtypes.bfloat16)
    )
    m1 = _build_m1()
    i127 = _build_i127()
    red = _build_red()

    in_maps = []
    for k in range(N_CORES):
        idxA = np.zeros((P, NROW * ICOL), dtype=np.int16)
        for r in range(NROW):
            idx = ip if r < B_LOC else is_
            row = idx[k * B_LOC + r % B_LOC]
            # slot g*128+p holds position 16p+g
            stream = np.zeros(NIDX, dtype=np.int16)
            stream[:S] = (row.reshape(P, NG).T.reshape(-1) - SPLIT).astype(np.int16)
            wrapped = np.tile(stream.reshape(-1, 16).T, (8, 1))
            idxA[:, r * ICOL : (r + 1) * ICOL] = wrapped
        in_maps.append(
            {"W_pri": wp, "W_sec": ws, "idxA": idxA, "m1": m1, "i127": i127, "red": red}
        )
    return in_maps


def _run(inputs_pri, inputs_sec, W_pri, W_sec, trace=False):
    global _last_results
    nc = _build_program()
    in_maps = _host_prep(inputs_pri, inputs_sec, W_pri, W_sec)
    res = run_bass_kernel_spmd(nc, in_maps, list(range(N_CORES)), trace=trace)
    _last_results = res
    out = np.empty((2, B, D), dtype=np.float32)
    for k in range(N_CORES):
        o = res.results[k]["out"]  # [16, 256]
        out[0, k * B_LOC : (k + 1) * B_LOC] = o[:B_LOC]
        out[1, k * B_LOC : (k + 1) * B_LOC] = o[B_LOC:]
    return out


def kernel(inputs_pri, inputs_sec, W_pri, W_sec):
    trace = bool(int(os.environ.get("KERNEL_TRACE", "0")))
    return _run(inputs_pri, inputs_sec, W_pri, W_sec, trace=trace)
